# revision 20
# baseline (speedup 1.0000x reference)
"""Trainium2 Bass kernel for GroupNorm(32) + single-head attention block.

Reference computation (per batch element b of 4, c=256, h=w=64, n=h*w=4096):
    xn = GroupNorm(32)(x) * gamma + beta
    q, k, v = split(W_qkv @ xn + b_qkv)          # 1x1 convs == channel matmuls
    S = (q^T k) / sqrt(c);  A = softmax(S);  o = A v
    out = W_out @ o + b_out + x
Sharding: 8 cores = 4 batch elements x 2 query-row halves. No collectives.

Implementation notes:
  - Every large matmul runs in fp8e4 DoubleRow mode (contract 256 in one
    instruction, 2 fp8 MACs/PE/cycle). Weights ship host-side pre-transposed
    and pre-scaled by 16 (so w~N(0,1) stays in fp8e4 normal range); the 1/16
    is folded into the psum evictions / final bias-add.
  - exp() runs with scale=1/sqrt(c) and bias=-2 folded into the activation:
    softmax is shift-invariant and e^(S-2) stays inside fp8e4's +-240 range.
    The attention output is rescaled by 64 before its fp8 cast (flat softmax
    keeps |o| small) and divided back in the final bias-add.
  - K/V/Q production is interleaved with block 0's S/exp stream so the ACT
    exp pipeline (the critical engine) starts as early as possible; psum
    psum evictions all run on DVE (GpSimd cannot access PSUM); the GN
    applies (SBUF->SBUF) run on GpSimd. V's bias is folded into b_out on
    the host (b_out + W_out @ b_v) so its eviction is a single multiply.
  - GroupNorm rstd uses a DVE-only Newton iteration; the scalar engine only
    ever loads the exp activation table, preloaded right after the PE warmup.
  - The attention loop is software-pipelined: per j-pair iteration of block
    ib we emit the S^T matmuls + exps of block ib+1, one deferred tail
    closure from block ib-1 (transpose / out-proj / store), and the DoubleRow
    PV accumulation matmuls of block ib.
"""

import numpy as np

import concourse.bass as bass
import concourse.tile as tile
from concourse import bacc, mybir
from concourse.bass_utils import run_bass_kernel_spmd
from concourse.masks import make_identity

P = 128
C = 256            # channels
N = 4096           # tokens per batch element (h*w)
H = 2048           # query rows per core (half of N)
CT = C // P        # 2 c-tiles
G = 32             # groups
GS = C // G        # 8 channels per group
GPT = P // GS      # 16 groups per c-tile
EPS = 1e-5
QSCALE = C ** -0.5
JT = N // P        # 32 key j-chunks
F32 = mybir.dt.float32
BF16 = mybir.dt.bfloat16
FP8 = mybir.dt.float8e4
DR = mybir.MatmulPerfMode.DoubleRow
VW = 272           # v_sb free width: 256 channels + ones col + pad to 16
EXPBIAS = -2.0     # shift exp() into fp8e4 range (softmax-invariant)
W16 = 1.0 / 16.0   # undo the host-side x16 fp8 weight scaling
AOP = mybir.AluOpType

_BUILD_CACHE = {}


def _build_nc():
    nc = bacc.Bacc()
    x_full = nc.declare_dram_parameter("x_full", [C, N], BF16, isOutput=False)
    x_half = nc.declare_dram_parameter("x_half", [C, H], BF16, isOutput=False)
    gn_gamma = nc.declare_dram_parameter("gn_gamma", [C], F32, isOutput=False)
    gn_beta = nc.declare_dram_parameter("gn_beta", [C], F32, isOutput=False)
    w_qkvT = nc.declare_dram_parameter("w_qkvT", [C, 3 * C], FP8, isOutput=False)
    b_qkv = nc.declare_dram_parameter("b_qkv", [3 * C], F32, isOutput=False)
    w_outT = nc.declare_dram_parameter("w_outT", [C, C], FP8, isOutput=False)
    b_out = nc.declare_dram_parameter("b_out", [C], F32, isOutput=False)
    out_ext = nc.declare_dram_parameter("out", [C, H], F32, isOutput=True)

    with tile.TileContext(nc) as tc:
        with (
            tc.tile_pool(name="consts", bufs=1) as consts,
            tc.tile_pool(name="acts", bufs=1) as acts,
            tc.tile_pool(name="stp", bufs=40) as stp,
            tc.tile_pool(name="smalls", bufs=2) as smalls,
            tc.tile_pool(name="tiny", bufs=8) as tiny,
            tc.tile_pool(name="stats", bufs=1) as stats_pool,
            tc.tile_pool(name="psS", bufs=3, space="PSUM") as psS,
            tc.tile_pool(name="psV", bufs=4, space="PSUM") as psV,
            tc.tile_pool(name="psT", bufs=1, space="PSUM") as psT,
        ):
            # ---------------- constants + loads ----------------
            ident_b = consts.tile([P, P], BF16)
            make_identity(nc, ident_b)

            gamma_p = consts.tile([P, CT], F32)
            nc.sync.dma_start(out=gamma_p, in_=gn_gamma[:].rearrange("(t p) -> p t", p=P))
            beta_p = consts.tile([P, CT], F32)
            nc.sync.dma_start(out=beta_p, in_=gn_beta[:].rearrange("(t p) -> p t", p=P))
            bqkv_p = consts.tile([P, 6], F32)
            nc.sync.dma_start(out=bqkv_p, in_=b_qkv[:].rearrange("(a p) -> p a", p=P))
            bout_p = consts.tile([P, CT], F32)
            nc.sync.dma_start(out=bout_p, in_=b_out[:].rearrange("(t p) -> p t", p=P))
            NQ = N // 4
            xf = [
                [acts.tile([P, NQ], BF16, name=f"xf{t}_{q}") for q in range(4)]
                for t in range(CT)
            ]
            xr = x_full[:].rearrange("(t p) n -> t p n", p=P)
            # 3 usable DMA queues (SP/ACT/gpsimd): x quarters spread so both
            # c-tiles of each quarter land about together
            qmap = {(0, 0): nc.sync, (1, 0): nc.scalar,
                    (0, 1): nc.scalar, (1, 1): nc.sync,
                    (0, 2): nc.sync, (1, 2): nc.scalar,
                    (0, 3): nc.gpsimd, (1, 3): nc.gpsimd}
            # gpsimd's pair first: its engine stream must fire these triggers
            # before any other gpsimd work
            for t in range(CT):
                qmap[(t, 3)].dma_start(
                    out=xf[t][3], in_=xr[t][:, 3 * NQ : 4 * NQ]
                )
            for q in range(3):
                for t in range(CT):
                    qmap[(t, q)].dma_start(
                        out=xf[t][q], in_=xr[t][:, q * NQ : (q + 1) * NQ]
                    )
            xh = [acts.tile([P, H], BF16, name=f"xh{t}") for t in range(CT)]
            xhr = x_half[:].rearrange("(t p) n -> t p n", p=P)
            nc.sync.dma_start(out=xh[0], in_=xhr[0])
            nc.scalar.dma_start(out=xh[1], in_=xhr[1])
            wq16 = consts.tile([P, CT, 3 * C], FP8)
            nc.gpsimd.dma_start(
                out=wq16, in_=w_qkvT[:].rearrange("(t p) o -> p t o", p=P)
            )
            woT = consts.tile([P, CT, C], FP8)
            nc.gpsimd.dma_start(
                out=woT, in_=w_outT[:].rearrange("(t p) o -> p t o", p=P)
            )
            wjunk = consts.tile([P, 512], BF16)
            nc.gpsimd.memset(wjunk, 0.001)
            expb = consts.tile([P, 1], F32)
            nc.gpsimd.memset(expb, EXPBIAS)

            # group-aggregation selector: sel[ch, g] = 1/GS if ch//GS == g
            sel = consts.tile([P, GPT], F32)
            nc.gpsimd.memset(sel, 1.0 / GS)
            nc.gpsimd.affine_select(
                out=sel, in_=sel, compare_op=AOP.is_ge, fill=0.0,
                base=0, pattern=[[-GS, GPT]], channel_multiplier=1,
            )
            nc.gpsimd.affine_select(
                out=sel, in_=sel, compare_op=AOP.is_ge, fill=0.0,
                base=GS - 1, pattern=[[GS, GPT]], channel_multiplier=-1,
            )
            # broadcast selector: bsel[g, ch] = 1 if ch//GS == g
            bsel = consts.tile([GPT, P], F32)
            nc.gpsimd.memset(bsel, 1.0)
            nc.gpsimd.affine_select(
                out=bsel, in_=bsel, compare_op=AOP.is_ge, fill=0.0,
                base=0, pattern=[[1, P]], channel_multiplier=-GS,
            )
            nc.gpsimd.affine_select(
                out=bsel, in_=bsel, compare_op=AOP.is_ge, fill=0.0,
                base=GS - 1, pattern=[[-1, P]], channel_multiplier=GS,
            )

            # PE warmup: consume the gpsimd-built constants so later PE
            # instructions never pair a fresh gpsimd wait with a data wait.
            warm = psT.tile([GPT, GPT], F32, tag="t128")
            nc.tensor.matmul(warm, lhsT=sel, rhs=sel, start=True, stop=True)
            warm2 = psT.tile([P, P], F32, tag="t128")
            nc.tensor.matmul(warm2, lhsT=bsel, rhs=bsel, start=True, stop=True)
            # preload the exp activation table (the only table this kernel
            # uses) long before the attention loop needs it
            dummy_exp = stats_pool.tile([GPT, 1], F32)
            exp_seed = stats_pool.tile([GPT, 1], F32)
            nc.vector.memset(exp_seed, 0.0)
            nc.scalar.activation(
                out=dummy_exp, in_=exp_seed, func=mybir.ActivationFunctionType.Exp
            )
            # keep the PE busy while GN stats wait on the x stream (cold PE
            # runs at half clock for ~3.4us of activity)
            for wi in range(20):
                jp_ = psS.tile([P, P], F32, tag="s", name=f"junk{wi}")
                nc.tensor.matmul(jp_, lhsT=ident_b, rhs=ident_b, start=True, stop=True)
            for wi in range(30):
                jp_ = psS.tile([P, 512], F32, tag="s", name=f"junkw{wi}")
                nc.tensor.matmul(jp_, lhsT=ident_b, rhs=wjunk, start=True, stop=True)

            # ---------------- GroupNorm statistics ----------------
            # c-tile 0: DVE bn_stats in 512-wide pieces as quarters land.
            # c-tile 1: ACT Square/Copy with free-dim accumulation (the
            # scalar engine is otherwise idle until the exp stream starts),
            # so neither engine backlogs behind the x DMA.
            ts2 = stats_pool.tile([P, CT, 2], F32)
            mv = stats_pool.tile([P, 2], F32)
            bstats = stats_pool.tile([P, 8, 6], F32)
            sq_scr = stats_pool.tile([P, NQ], BF16)
            cp_acc = stats_pool.tile([P, 4], F32)
            sq_acc = stats_pool.tile([P, 4], F32)
            for q in range(4):
                for s in range(2):
                    nc.vector.bn_stats(
                        out=bstats[:, 2 * q + s, :],
                        in_=xf[0][q][:, s * 512 : (s + 1) * 512],
                    )
                nc.scalar.activation(
                    out=sq_scr, in_=xf[1][q],
                    func=mybir.ActivationFunctionType.Square,
                    accum_out=sq_acc[:, q : q + 1],
                )
                nc.scalar.activation(
                    out=sq_scr, in_=xf[1][q],
                    func=mybir.ActivationFunctionType.Copy,
                    accum_out=cp_acc[:, q : q + 1],
                )
            nc.vector.bn_aggr(out=mv, in_=bstats)
            # ts2 col0 = mean, col1 = E[x^2]
            nc.vector.tensor_copy(out=ts2[:, 0, 0:1], in_=mv[:, 0:1])
            nc.vector.tensor_mul(ts2[:, 0, 1:2], mv[:, 0:1], mv[:, 0:1])
            nc.vector.tensor_add(ts2[:, 0, 1:2], ts2[:, 0, 1:2], mv[:, 1:2])
            acc2 = stats_pool.tile([P, 2], F32)
            nc.vector.tensor_reduce(
                out=acc2[:, 0:1], in_=cp_acc, axis=mybir.AxisListType.X,
                op=AOP.add,
            )
            nc.vector.tensor_reduce(
                out=acc2[:, 1:2], in_=sq_acc, axis=mybir.AxisListType.X,
                op=AOP.add,
            )
            nc.vector.tensor_scalar(
                out=ts2[:, 1, :], in0=acc2, scalar1=1.0 / N, scalar2=None,
                op0=AOP.mult,
            )

            # aggregate channels -> groups:  gv[g, t] = (M_g, E2_g)
            gv = stats_pool.tile([GPT, CT, 2], F32)
            gp = psT.tile([GPT, CT * 2], F32, tag="t128")
            nc.tensor.matmul(
                gp, lhsT=sel, rhs=ts2.rearrange("p t c -> p (t c)"),
                start=True, stop=True,
            )
            nc.vector.tensor_copy(out=gv.rearrange("g t c -> g (t c)"), in_=gp)

            # more PE filler while the DVE runs the Newton/scale chain below
            for wi in range(14):
                jp_ = psS.tile([P, 512], F32, tag="s", name=f"junkn{wi}")
                nc.tensor.matmul(jp_, lhsT=ident_b, rhs=wjunk, start=True, stop=True)

            # rstd_g = rsqrt(E2 - M^2 + eps) via DVE-only Newton iteration
            # (seeded at 1.0: inputs are ~unit-variance). y <- y*(1.5-0.5*v*y^2)
            gAB = stats_pool.tile([GPT, CT, 2], F32)  # col0 = M_g, col1 = rstd_g
            vv = stats_pool.tile([GPT, CT], F32)
            nc.vector.tensor_mul(vv, gv[:, :, 0], gv[:, :, 0])
            nc.vector.tensor_tensor(out=vv, in0=gv[:, :, 1], in1=vv, op=AOP.subtract)
            nc.vector.tensor_scalar(
                out=vv, in0=vv, scalar1=float(EPS), scalar2=-0.5,
                op0=AOP.add, op1=AOP.mult,
            )  # vv holds -0.5*(var+eps)
            y = stats_pool.tile([GPT, CT], F32)
            nc.vector.memset(y, 1.0)
            t1 = stats_pool.tile([GPT, CT], F32)
            for _ in range(2):
                nc.vector.tensor_mul(t1, y, y)              # y^2
                nc.vector.tensor_mul(t1, t1, vv)            # -0.5*v*y^2
                nc.vector.tensor_scalar(
                    out=t1, in0=t1, scalar1=1.5, scalar2=None, op0=AOP.add
                )                                           # 1.5 - 0.5*v*y^2
                nc.vector.tensor_mul(y, y, t1)
            nc.vector.tensor_copy(out=gAB[:, :, 0], in_=gv[:, :, 0])
            nc.vector.tensor_copy(out=gAB[:, :, 1], in_=y)

            # broadcast groups -> channels; per-channel scale/shift
            scale_sb = stats_pool.tile([P, CT, 1], F32)
            shift_sb = stats_pool.tile([P, CT, 1], F32)
            bp = psT.tile([P, CT * 2], F32, tag="t128")
            nc.tensor.matmul(
                bp, lhsT=bsel, rhs=gAB.rearrange("g t c -> g (t c)"),
                start=True, stop=True,
            )
            chMR = stats_pool.tile([P, CT, 2], F32)
            nc.vector.tensor_copy(out=chMR, in_=bp)
            # scale = gamma * rstd ; shift = beta - mean * scale
            nc.vector.tensor_mul(scale_sb[:, :, 0], gamma_p, chMR[:, :, 1])
            nc.vector.tensor_mul(shift_sb[:, :, 0], chMR[:, :, 0], scale_sb[:, :, 0])
            nc.vector.tensor_tensor(
                out=shift_sb[:, :, 0], in0=beta_p, in1=shift_sb[:, :, 0],
                op=AOP.subtract,
            )

            # -------- normalized activations (fp8) + QKV projections -------
            xnh8 = acts.tile([P, CT, H], FP8)
            xn8 = acts.tile([P, CT, N], FP8)
            q_sb = acts.tile([P, CT, H], FP8)
            k_sb = acts.tile([P, CT, N], FP8)
            v_sb = acts.tile([P, JT, VW], FP8)
            nc.gpsimd.memset(v_sb[:, :, C:], 0.0)
            nc.gpsimd.memset(v_sb[:, :, C : C + 1], 1.0)

            def xnh8_apply(ib, eng=None):
                eng = eng or nc.gpsimd
                for t in range(CT):
                    eng.tensor_scalar(
                        out=xnh8[:, t, ib * 512 : (ib + 1) * 512],
                        in0=xh[t][:, ib * 512 : (ib + 1) * 512],
                        scalar1=scale_sb[:, t, :], scalar2=shift_sb[:, t, :],
                        op0=AOP.mult, op1=AOP.add,
                    )

            def xn8_apply(qr):  # gpsimd (SBUF->SBUF)
                for t in range(CT):
                    nc.gpsimd.tensor_scalar(
                        out=xn8[:, t, qr * NQ : (qr + 1) * NQ], in0=xf[t][qr],
                        scalar1=scale_sb[:, t, :], scalar2=shift_sb[:, t, :],
                        op0=AOP.mult, op1=AOP.add,
                    )

            def q_proj(ib):
                for ot in range(CT):
                    qp = psS.tile([P, 512], F32, tag="s", name=f"qp{ib}_{ot}")
                    nc.tensor.matmul(
                        qp, lhsT=wq16[:, :, ot * P : (ot + 1) * P],
                        rhs=xnh8[:, :, ib * 512 : (ib + 1) * 512],
                        start=True, stop=True, perf_mode=DR,
                    )
                    nc.vector.tensor_scalar(
                        out=q_sb[:, ot, ib * 512 : (ib + 1) * 512], in0=qp,
                        scalar1=W16, scalar2=bqkv_p[:, ot, None],
                        op0=AOP.mult, op1=AOP.add,
                    )

            def k_proj(jb):
                for ot in range(CT):
                    kp = psS.tile([P, 512], F32, tag="s", name=f"kp{jb}_{ot}")
                    nc.tensor.matmul(
                        kp, lhsT=wq16[:, :, C + ot * P : C + (ot + 1) * P],
                        rhs=xn8[:, :, jb * 512 : (jb + 1) * 512],
                        start=True, stop=True, perf_mode=DR,
                    )
                    nc.vector.tensor_scalar(
                        out=k_sb[:, ot, jb * 512 : (jb + 1) * 512], in0=kp,
                        scalar1=W16, scalar2=bqkv_p[:, 2 + ot, None],
                        op0=AOP.mult, op1=AOP.add,
                    )

            def v_proj(jp):
                # both chunks of the j-pair into one psum bank, one eviction
                vp = psV.tile([P, 2 * C], F32, tag="v", name=f"vp{jp}")
                for hh in range(2):
                    jt = 2 * jp + hh
                    nc.tensor.matmul(
                        vp[:, hh * C : (hh + 1) * C],
                        lhsT=xn8[:, :, jt * P : (jt + 1) * P],
                        rhs=wq16[:, :, 2 * C : 3 * C],
                        start=True, stop=True, perf_mode=DR,
                    )
                nc.vector.tensor_scalar(
                    out=v_sb[:, 2 * jp : 2 * jp + 2, :C],
                    in0=vp.rearrange("p (hh c) -> p hh c", hh=2),
                    scalar1=W16, scalar2=None, op0=AOP.mult,
                )

            out_r = out_ext[:].rearrange("(t p) n -> p t n", p=P)
            EXPF = mybir.ActivationFunctionType.Exp
            # last 512 i-block split in two so the forced-serial final tail
            # (evict/transpose/proj/store after the last PV) is half-size
            blocks = [(0, 512), (512, 512), (1024, 512), (1536, 384), (1920, 128)]

            def emit_s(bi, jp, sts):
                # one j-pair: two fp8 DoubleRow S^T matmuls (contract all 256
                # channels at once) + two exps into the paired fp8 tile that
                # feeds the DoubleRow PV lhsT
                i0, w = blocks[bi]
                st2 = stp.tile([P, 2, w], FP8, tag="st", name=f"st_{bi}_{jp}")
                for h in range(2):
                    jt = 2 * jp + h
                    sp = psS.tile([P, w], F32, tag="s", name=f"sp_{bi}_{jt}")
                    nc.tensor.matmul(
                        sp,
                        lhsT=k_sb[:, :, jt * P : (jt + 1) * P],
                        rhs=q_sb[:, :, i0 : i0 + w],
                        start=True, stop=True, perf_mode=DR,
                    )
                    nc.scalar.activation(
                        out=st2[:, h, :], in_=sp, func=EXPF,
                        scale=float(QSCALE), bias=expb,
                    )
                sts.append(st2)

            # -------- prefill: K/V/Q production rides block 0's S stream ---
            st_blocks = {0: []}
            xnh8_apply(0, nc.vector)  # DVE: gpsimd is busy with xn8(0)
            xn8_apply(0)
            q_proj(0)
            extras = []
            for ib in range(1, 4):
                extras.append(lambda ib=ib: xnh8_apply(ib))
                extras.append(lambda ib=ib: q_proj(ib))
            PRE1 = 7   # block-1 j-pairs emitted during prefill (ACT gap fill)
            st_blocks[1] = []
            for jp in range(JT // 2):
                if jp % 2 == 0:
                    k_proj(jp // 2)
                elif jp % 4 == 1 and jp // 4 + 1 < 4:
                    xn8_apply(jp // 4 + 1)
                elif extras:
                    extras.pop(0)()
                emit_s(0, jp, st_blocks[0])
                v_proj(jp)
                if jp >= JT // 2 - PRE1:
                    emit_s(1, jp - (JT // 2 - PRE1), st_blocks[1])
            while extras:
                extras.pop(0)()

            # ---------------- attention + output projection ----------------
            pending = []

            def make_tail(bi, pvs):
                i0, w = blocks[bi]
                nsub = w // P
                aoT = smalls.tile([P, CT, w], FP8, tag="aoT", name=f"aoT{bi}")
                ao_list = []

                def evict(isub):
                    def _f():
                        pv = pvs[isub]
                        rsum = tiny.tile([P, 1], F32, tag="rsum")
                        nc.vector.reciprocal(out=rsum, in_=pv[:, C : C + 1])
                        ao = tiny.tile([P, C], BF16, tag="ao")
                        nc.vector.tensor_scalar(
                            out=ao, in0=pv[:, :C], scalar1=rsum, scalar2=64.0,
                            op0=AOP.mult, op1=AOP.mult,
                        )
                        ao_list.append(ao)
                    return _f

                def transp(isub, t):
                    def _f():
                        tp = psT.tile([P, P], BF16, tag="t128")
                        nc.tensor.transpose(
                            tp, ao_list[isub][:, t * P : (t + 1) * P], ident_b
                        )
                        nc.vector.tensor_copy(
                            out=aoT[:, t, isub * P : (isub + 1) * P], in_=tp
                        )
                    return _f

                out_sb = smalls.tile([P, CT, w], F32, tag="out_sb", name=f"osb{bi}")

                def proj(ot):
                    def _f():
                        op = psT.tile([P, w], F32, tag="t128")
                        nc.tensor.matmul(
                            op,
                            lhsT=woT[:, :, ot * P : (ot + 1) * P],
                            rhs=aoT[:, :, :],
                            start=True, stop=True, perf_mode=DR,
                        )
                        nc.vector.tensor_scalar(
                            out=out_sb[:, ot, :], in0=op,
                            scalar1=1.0 / 1024.0, scalar2=bout_p[:, ot, None],
                            op0=AOP.mult, op1=AOP.add,
                        )
                        nc.vector.tensor_add(
                            out_sb[:, ot, :], out_sb[:, ot, :],
                            xh[ot][:, i0 : i0 + w],
                        )
                    return _f

                def store():
                    # split across both DMA queues: the final store is on the
                    # kernel's drain-critical path
                    nc.sync.dma_start(
                        out=out_r[:, 0, i0 : i0 + w], in_=out_sb[:, 0, :]
                    )
                    nc.scalar.dma_start(
                        out=out_r[:, 1, i0 : i0 + w], in_=out_sb[:, 1, :]
                    )

                fs = []
                for isub in range(nsub):
                    fs.append(evict(isub))
                    fs.append(transp(isub, 0))
                    fs.append(transp(isub, 1))
                fs.append(proj(0))
                fs.append(proj(1))
                fs.append(store)
                return fs

            prev_nsub = 0
            for bi in range(len(blocks)):
                nxt = bi + 1
                if nxt < len(blocks) and nxt not in st_blocks:
                    st_blocks[nxt] = []
                sts = st_blocks[bi]
                nsub = blocks[bi][1] // P
                # flush the previous block's PV evictions first so its psum
                # slots are released for this block's accumulators
                for _ in range(min(len(pending), prev_nsub)):
                    pending.pop(0)()
                pvs = [
                    psV.tile([P, VW], F32, tag="v", name=f"pv{bi}_{isub}")
                    for isub in range(nsub)
                ]
                NJP = JT // 2
                done = len(st_blocks.get(nxt, ())) if nxt < len(blocks) else NJP
                for jp in range(NJP):
                    if nxt < len(blocks) and jp >= done:
                        emit_s(nxt, jp, st_blocks[nxt])
                    if pending:
                        pending.pop(0)()
                    for isub in range(nsub):
                        nc.tensor.matmul(
                            pvs[isub],
                            lhsT=sts[jp][:, :, isub * P : (isub + 1) * P],
                            rhs=v_sb[:, 2 * jp : 2 * jp + 2, :],
                            start=(jp == 0), stop=(jp == NJP - 1),
                            skip_group_check=True, perf_mode=DR,
                        )
                pending.extend(make_tail(bi, pvs))
                del st_blocks[bi]
                prev_nsub = nsub
            while pending:
                pending.pop(0)()

    nc.finalize()
    return nc


def kernel(x, gn_gamma, gn_beta, w_qkv, b_qkv, w_out, b_out, _trace=False):
    import kernel as _self

    b, c, h, w = x.shape
    assert (b, c, h, w) == (4, 256, 64, 64)
    x = np.ascontiguousarray(np.asarray(x, dtype=np.float32))

    if "nc" not in _BUILD_CACHE:
        _BUILD_CACHE["nc"] = _build_nc()
    nc = _BUILD_CACHE["nc"]

    import ml_dtypes

    w_qkvT = np.ascontiguousarray(
        (np.asarray(w_qkv, np.float32).T * 16.0).astype(ml_dtypes.float8_e4m3)
    )
    w_outT = np.ascontiguousarray(
        (np.asarray(w_out, np.float32).T * 16.0).astype(ml_dtypes.float8_e4m3)
    )
    # v's bias is dropped on-device; W_out @ b_v folds into the output bias
    b_out_eff = np.asarray(b_out, np.float32) + np.asarray(
        w_out, np.float32
    ) @ np.asarray(b_qkv, np.float32)[2 * C : 3 * C]
    x_bf = x.astype(ml_dtypes.bfloat16)
    in_maps = []
    for core in range(8):
        bi, hi = core // 2, core % 2
        in_maps.append(
            {
                "x_full": x_bf[bi].reshape(C, N),
                "x_half": np.ascontiguousarray(
                    x_bf[bi, :, 32 * hi : 32 * hi + 32, :]
                ).reshape(C, H),
                "gn_gamma": np.asarray(gn_gamma, np.float32),
                "gn_beta": np.asarray(gn_beta, np.float32),
                "w_qkvT": w_qkvT,
                "b_qkv": np.asarray(b_qkv, np.float32),
                "w_outT": w_outT,
                "b_out": b_out_eff,
            }
        )

    res = run_bass_kernel_spmd(nc, in_maps, core_ids=list(range(8)), trace=_trace)
    _self._LAST_RESULT = res

    out = np.empty((b, c, h, w), dtype=np.float32)
    for core in range(8):
        bi, hi = core // 2, core % 2
        out[bi, :, 32 * hi : 32 * hi + 32, :] = res.results[core]["out"].reshape(
            C, 32, 64
        )
    return out
